# revision 35
# baseline (speedup 1.0000x reference)
"""Trainium2 Bass kernel: multi-head causal attention (B=4, T=2048, C=1024, H=16, HS=64).

Sharding: hybrid batch x head tensor-parallel over 8 cores.
  core c -> batch b = c//2, head half p = c%2 (heads p*8 .. p*8+8).
Each core computes Q/K/V projections for its 8 heads on its batch and causal
flash-style attention (scores transposed: s on partitions, t on free dim,
softmax denominator via a ones-column appended to V).  The attention outputs
(ao, bf16) are exchanged between the two cores of a batch pair with four
small per-head-pair AllToAlls (each core keeps the t-half it will project),
then each core runs the full-contraction output projection for its T/2 rows.
No ReduceScatter of f32 partials is needed.

Perf notes vs the earlier RS-based version:
  - reciprocal batched over 8 denominator rows at once ([8,512] costs the
    same as [1,512]; DVE RECIPROCAL is ~8 cycles/element along the free dim)
  - exp batched in 2-PSUM-bank pairs (amortizes the ~352-cycle ACT ramp)
  - qt/kt half duplication (for PE row tiling) done by SBUF->SBUF DMA
    instead of extra DVE copies
  - causal masks for a chunk pair applied in one tensor_tensor op against a
    host-precomputed [P, 4, 512] mask
"""

import os
import sys
import time

import numpy as np

for _p in ("/opt/trn_rl_repo", "/root/.axon_site/_ro/trn_rl_repo"):
    if os.path.isdir(_p) and _p not in sys.path:
        sys.path.insert(0, _p)

import ml_dtypes  # noqa: E402
import concourse.bass as bass  # noqa: E402,F401
import concourse.mybir as mybir  # noqa: E402
import concourse.tile as tile  # noqa: E402
from concourse import bacc  # noqa: E402
from concourse.bass_utils import run_bass_kernel_spmd  # noqa: E402

B, T, C, H, HS = 4, 2048, 1024, 16, 64
N_CORES = 8
NH = H // 2          # heads per core
P = 128
TJ = 512             # t-tile width for attention
NTJ = T // TJ        # 4
NSK = T // P         # 16 s-chunks
NCK = C // P         # 8 contraction chunks
TH = T // 2          # rows output per core
BF16 = mybir.dt.bfloat16
F32 = mybir.dt.float32
GROUPS = [[0, 1], [2, 3], [4, 5], [6, 7]]

# Schraudolph bf16-exp constants: int16(bf16 bits of exp(score/sqrt(HS)))
# ~= score * (2^7 * log2(e) / sqrt(HS)) + (127*2^7 - 5.58)
_SCH_A = 128.0 * 1.4426950408889634 / 8.0
_SCH_B = 16250.4

_NC_CACHE = {}


def build_nc(with_collective=True, stages=frozenset(
        {"proj", "scores", "exp", "mask", "av", "norm", "outproj", "cc"}),
        repeat=1, sp_bufs=2, avp_bufs=2, dup_dma=True, dve_every=0):
    key = (with_collective, tuple(sorted(stages)), repeat, sp_bufs, avp_bufs,
           dup_dma, dve_every)
    if key in _NC_CACHE:
        return _NC_CACHE[key]
    en = stages.__contains__
    nc = bacc.Bacc("TRN2", target_bir_lowering=False, debug=False,
                   num_devices=N_CORES)
    xb_d = nc.dram_tensor("xb", [P, NCK, T], BF16, kind="ExternalInput")
    wqk_d = nc.dram_tensor("wqk", [P, NH, NCK, P], BF16, kind="ExternalInput")
    wv_d = nc.dram_tensor("wv", [P, NCK, NH * HS], BF16, kind="ExternalInput")
    wot_d = nc.dram_tensor("wot", [P, 8, C], BF16, kind="ExternalInput")
    bo_d = nc.dram_tensor("bo_bc", [P, C], F32, kind="ExternalInput")
    mask_d = nc.dram_tensor("maskp", [P, 4, TJ], BF16, kind="ExternalInput")
    use_cc = with_collective and en("cc")
    if use_cc:
        ccin = [nc.dram_tensor(f"ccin{pj}", [P, T], BF16)
                for pj in range(4)]
        ccout = [nc.dram_tensor(f"ccout{pj}", [2 * P, T], BF16)
                 for pj in range(4)]
    out_d = nc.dram_tensor("out", [TH, C], F32, kind="ExternalOutput")
    if en("dbg"):
        dbg_ao = nc.dram_tensor("dbg_ao", [P, 4, T], BF16,
                                kind="ExternalOutput")
        dbg_aop = nc.dram_tensor("dbg_aop", [P, 2, 4, TH], BF16,
                                 kind="ExternalOutput")

    from contextlib import ExitStack
    with tile.TileContext(nc) as tc, ExitStack() as ctx:
        const = ctx.enter_context(tc.tile_pool(name="const", bufs=1))
        qk_pool = ctx.enter_context(tc.tile_pool(name="qk", bufs=2))
        attn_pool = ctx.enter_context(tc.tile_pool(name="attn", bufs=2))
        den_pool = ctx.enter_context(tc.tile_pool(name="den", bufs=2))
        stash_pool = ctx.enter_context(tc.tile_pool(name="stash", bufs=1))
        outs_pool = ctx.enter_context(tc.tile_pool(name="outs", bufs=2))
        pp = ctx.enter_context(tc.tile_pool(name="pp", bufs=2, space="PSUM"))
        sp = ctx.enter_context(tc.tile_pool(name="sp", bufs=sp_bufs, space="PSUM"))
        avp = ctx.enter_context(tc.tile_pool(name="avp", bufs=avp_bufs, space="PSUM"))

        # ---- constants / persistent tiles ----
        wqk_sb = const.tile([P, NH, NCK, P], BF16)
        wv_sb = const.tile([P, NCK, NH * HS], BF16)
        wot_sb = const.tile([P, 8, C], BF16)
        bo_sb = const.tile([P, C], F32)
        xb = const.tile([P, NCK, T], BF16)          # x^T bf16 (c, t)
        v1 = const.tile([P, NH, NSK, HS + 1], BF16)  # [s, head, sk, d|1]
        ao = const.tile([P, 4, T], BF16)            # normalized attn out^T
        aop = const.tile([P, 2, 4, TH], BF16)       # A2A result (blk, pj, t)
        mask_sb = const.tile([P, 4, TJ], BF16)
        for i4 in range(4):
            nc.sync.dma_start(out=wqk_sb[:, 2 * i4:2 * i4 + 2, :, :],
                              in_=wqk_d[:, 2 * i4:2 * i4 + 2, :, :])
            nc.sync.dma_start(out=wot_sb[:, 2 * i4:2 * i4 + 2, :],
                              in_=wot_d[:, 2 * i4:2 * i4 + 2, :])
        for i2 in range(2):
            nc.sync.dma_start(out=wv_sb[:, 4 * i2:4 * i2 + 4, :],
                              in_=wv_d[:, 4 * i2:4 * i2 + 4, :])
        nc.sync.dma_start(out=bo_sb[:], in_=bo_d[:])
        nc.sync.dma_start(out=mask_sb[:], in_=mask_d[:])
        nc.vector.memset(v1[:, :, :, HS:HS + 1], 1.0)

        if use_cc:
            # rank within the pair, as a sync-engine register for
            # predicated (cond=) DMAs -- the program is SPMD, so any
            # rank-dependent addressing must be runtime-predicated.
            _pid = nc.sync.partition_id()
            r_reg = _pid % 2
            nr_reg = 1 - r_reg

        for _rep in range(repeat):

            # ---- load x^T (bf16, split for DMA-engine parallelism) ----
            for h in range(2):
                for ck in range(NCK):
                    nc.sync.dma_start(
                        out=xb[:, ck, h * TH:(h + 1) * TH],
                        in_=xb_d[:, ck, h * TH:(h + 1) * TH])

            # ---- V projection (all heads at once; ACT does the PSUM
            # evacuation -- it is idle until the first scores arrive) ----
            for sk in range(NSK if en("proj") else 0):
                ps = pp.tile([P, NH * HS], F32, tag="pp")
                for ck in range(NCK):
                    nc.tensor.matmul(
                        ps[:], xb[:, ck, sk * P:(sk + 1) * P], wv_sb[:, ck, :],
                        start=(ck == 0), stop=(ck == NCK - 1))
                nc.scalar.copy(
                    v1[:, :, sk, 0:HS],
                    ps[:].rearrange("p (i d) -> p i d", d=HS))

            # ---- per-head attention, software-pipelined ----
            # PE is in-order; scores matmuls are paced by ACT exp draining
            # PSUM slots.  To keep PE busy during those waits, the previous
            # block's AV matmuls (which read SBUF attn tiles and their own
            # PSUM bank) are interleaved between scores/proj matmuls.
            def make_av_thunks(i, tj, attn, stash):
                if not en("av"):
                    return []
                pj = i // 2
                half = (i % 2) * HS
                n_sk = 4 * (tj + 1)
                av = avp.tile([P, TJ], F32, tag="avp")

                def mm(sk):
                    nc.tensor.matmul(av[0:HS + 1, :], v1[:, i, sk, :],
                                     attn[:, sk, :], start=(sk == 0),
                                     stop=(sk == n_sk - 1))
                thunks = [lambda sk=sk: mm(sk) for sk in range(n_sk)]

                def evac():
                    if not en("norm"):
                        return
                    if en("oldnorm"):
                        recip = den_pool.tile([1, TJ], F32, tag="recip1")
                        nc.vector.reciprocal(recip[:], av[HS:HS + 1, :])
                        den_bc = den_pool.tile([HS, TJ], F32, tag="den_bc0")
                        nc.gpsimd.partition_broadcast(den_bc[:], recip[:])
                        nc.vector.tensor_tensor(
                            out=ao[half:half + HS, pj, tj * TJ:(tj + 1) * TJ],
                            in0=av[0:HS, :], in1=den_bc[:],
                            op=mybir.AluOpType.mult)
                        return
                    # unnormalized attention output + denominator row stash
                    # (packed on the free dim of a partition-0 tile; DVE and
                    # gpsimd only reliably address partition bases 0/64)
                    nc.vector.tensor_copy(
                        ao[half:half + HS, pj, tj * TJ:(tj + 1) * TJ],
                        av[0:HS, :])
                    nc.vector.tensor_copy(
                        stash[0:1, (i % 2) * NTJ + tj, :],
                        av[HS:HS + 1, :])
                thunks.append(evac)
                return thunks

            def make_norm_thunks(pj, stash):
                # after a head pair: spread the eight stashed denominator
                # rows across partitions via DMA, one batched reciprocal,
                # then scale ao in place.  Only partition bases 0/64 are
                # used for compute-engine APs (others are unreliable on HW).
                if not en("norm") or en("oldnorm"):
                    return []
                thunks = []
                st2d = den_pool.tile([32, TJ], BF16, tag="st2d")
                recip8 = den_pool.tile([32, TJ], BF16, tag="recip")

                def recip_all():
                    nc.sync.dma_start(out=st2d[0:2 * NTJ, :],
                                      in_=stash[0:1, :, :])
                    with nc.allow_low_precision(
                            reason="bf16 softmax denominators"):
                        nc.vector.reciprocal(recip8[:], st2d[:])
                thunks.append(recip_all)

                def norm_one(hh, tj):
                    half = hh * HS
                    tjsl = slice(tj * TJ, (tj + 1) * TJ)
                    r1 = den_pool.tile([1, TJ], BF16, tag="r1")
                    nc.sync.dma_start(
                        out=r1[:], in_=recip8[hh * NTJ + tj:hh * NTJ + tj + 1, :])
                    den_bc = den_pool.tile([P, TJ], BF16, tag="den_bc")
                    nc.gpsimd.partition_broadcast(den_bc[:], r1[:])
                    nc.vector.tensor_tensor(
                        out=ao[half:half + HS, pj, tjsl],
                        in0=ao[half:half + HS, pj, tjsl],
                        in1=den_bc[half:half + HS, :],
                        op=mybir.AluOpType.mult)
                for hh in range(2):
                    for tj in range(NTJ):
                        thunks.append(
                            lambda hh=hh, tj=tj: norm_one(hh, tj))
                return thunks

            def make_fin_thunks(pj):
                # after both heads of pair pj: exchange halves with the pair
                thunks = []
                if use_cc:
                    def exchange():
                        nc.sync.dma_start(out=ccin[pj][:], in_=ao[:, pj, :])
                        nc.gpsimd.collective_compute(
                            "AllGather", mybir.AluOpType.bypass,
                            replica_groups=GROUPS,
                            ins=[ccin[pj][:]], outs=[ccout[pj][:]])
                        # each rank keeps the t-half it will project
                        for blk in range(2):
                            rows = slice(blk * P, (blk + 1) * P)
                            if en("recvlo"):
                                nc.sync.dma_start(
                                    out=aop[:, blk, pj, :],
                                    in_=ccout[pj][rows, 0:TH])
                            else:
                                nc.sync.dma_start(
                                    out=aop[:, blk, pj, :],
                                    in_=ccout[pj][rows, 0:TH], cond=nr_reg)
                                nc.sync.dma_start(
                                    out=aop[:, blk, pj, :],
                                    in_=ccout[pj][rows, TH:T], cond=r_reg)
                    thunks.append(exchange)
                return thunks

            def emit_block(emitters, pending):
                L_s, L_a = len(emitters), len(pending)
                j = 0
                for k, e in enumerate(emitters):
                    e()
                    jt = L_a * (k + 1) // L_s if L_s else L_a
                    while j < jt:
                        pending[j]()
                        j += 1
                for t in pending[j:]:
                    t()

            pending = []
            _gp = [0]
            stash = None
            for i in range(NH):
                if i % 2 == 0:
                    stash = stash_pool.tile([1, 2 * NTJ, TJ], BF16,
                                            tag="stash")
                qt2 = qk_pool.tile([P, T], BF16, tag="qt2")
                kt2 = qk_pool.tile([P, T], BF16, tag="kt2")

                proj_emitters = []
                if en("proj"):
                    for tj in range(NTJ):
                        ps = pp.tile([P, TJ], F32, tag="pp")

                        def pmm(i=i, tj=tj, ps=ps, ck=None):
                            nc.tensor.matmul(
                                ps[:], wqk_sb[:, i, ck, :],
                                xb[:, ck, tj * TJ:(tj + 1) * TJ],
                                start=(ck == 0), stop=(ck == NCK - 1))

                        def pevac(i=i, tj=tj, ps=ps, qt2=qt2, kt2=kt2):
                            sl = slice(tj * TJ, (tj + 1) * TJ)
                            nc.vector.tensor_copy(qt2[0:HS, sl], ps[0:HS, :])
                            nc.vector.tensor_copy(kt2[HS:P, sl], ps[HS:P, :])
                            if dup_dma:
                                nc.sync.dma_start(out=qt2[HS:P, sl],
                                                  in_=qt2[0:HS, sl])
                                nc.sync.dma_start(out=kt2[0:HS, sl],
                                                  in_=kt2[HS:P, sl])
                            else:
                                nc.vector.tensor_copy(qt2[HS:P, sl],
                                                      ps[0:HS, :])
                                nc.vector.tensor_copy(kt2[0:HS, sl],
                                                      ps[HS:P, :])
                        for ck in range(NCK):
                            proj_emitters.append(
                                lambda f=pmm, ck=ck: f(ck=ck))
                        proj_emitters.append(pevac)
                emit_block(proj_emitters, pending)
                pending = []

                for tj in range(NTJ):
                    n_sk = 4 * (tj + 1)
                    attn = attn_pool.tile([P, n_sk, TJ], BF16, tag="attn")
                    sc_emitters = []
                    if en("nopair"):
                        for sk in range(n_sk if en("scores") else 0):
                            def smm1(i=i, tj=tj, sk=sk, attn=attn):
                                h0 = HS * (sk % 2)
                                sps = sp.tile([P, TJ], F32, tag="sp")
                                nc.tensor.matmul(
                                    sps[:],
                                    kt2[h0:h0 + HS, sk * P:(sk + 1) * P],
                                    qt2[h0:h0 + HS, tj * TJ:(tj + 1) * TJ],
                                    start=True, stop=True)
                                if en("exp"):
                                    nc.scalar.activation(
                                        attn[:, sk, :], sps[:],
                                        mybir.ActivationFunctionType.Exp,
                                        scale=1.0 / np.sqrt(HS))
                                kdiag = sk - 4 * tj
                                if kdiag >= 0 and en("mask"):
                                    nc.vector.tensor_tensor(
                                        out=attn[:, sk, :],
                                        in0=attn[:, sk, :],
                                        in1=mask_sb[:, kdiag, :],
                                        op=mybir.AluOpType.mult)
                            sc_emitters.append(smm1)
                    else:
                        # scores in pairs sharing one 2-bank PSUM tile; one
                        # exp per pair (amortizes ACT ramp); the diagonal
                        # region (last 4 chunks) gets one merged mask mult.
                        # A fraction of exps runs on DVE via the Schraudolph
                        # bit trick to offload the saturated ACT engine.
                        for p2 in range(n_sk // 2 if en("scores") else 0):
                            sk0 = 2 * p2
                            sps = sp.tile([P, 2, TJ], F32, tag="sp")
                            _gp[0] += 1
                            on_dve = dve_every and (_gp[0] % dve_every == 0)

                            def smm2(i=i, tj=tj, sk0=sk0, sps=sps):
                                for j in range(2):
                                    sk = sk0 + j
                                    h0 = HS * (sk % 2)
                                    nc.tensor.matmul(
                                        sps[:, j, :],
                                        kt2[h0:h0 + HS, sk * P:(sk + 1) * P],
                                        qt2[h0:h0 + HS,
                                            tj * TJ:(tj + 1) * TJ],
                                        start=True, stop=True)

                            def sexp(i=i, tj=tj, sk0=sk0, sps=sps, attn=attn,
                                     on_dve=on_dve):
                                if en("exp"):
                                    if on_dve:
                                        # bf16(exp(x*scale)) bit pattern as
                                        # int16 affine of the raw score
                                        nc.vector.tensor_scalar(
                                            out=attn[:, sk0:sk0 + 2, :]
                                            .bitcast(mybir.dt.int16),
                                            in0=sps[:],
                                            scalar1=_SCH_A, scalar2=_SCH_B,
                                            op0=mybir.AluOpType.mult,
                                            op1=mybir.AluOpType.add)
                                    else:
                                        nc.scalar.activation(
                                            attn[:, sk0:sk0 + 2, :], sps[:],
                                            mybir.ActivationFunctionType.Exp,
                                            scale=1.0 / np.sqrt(HS))
                                if sk0 == 4 * tj + 2 and en("mask"):
                                    nc.vector.tensor_tensor(
                                        out=attn[:, 4 * tj:4 * tj + 4, :],
                                        in0=attn[:, 4 * tj:4 * tj + 4, :],
                                        in1=mask_sb[:],
                                        op=mybir.AluOpType.mult)
                            sc_emitters.append(smm2)
                            sc_emitters.append(sexp)
                    emit_block(sc_emitters, pending)
                    pending = make_av_thunks(i, tj, attn, stash)
                if i % 2 == 1:
                    pending = (pending + make_norm_thunks(i // 2, stash)
                               + make_fin_thunks(i // 2))
            for t in pending:
                t()
            pending = []

            if en("dbg"):
                nc.sync.dma_start(out=dbg_ao[:], in_=ao[:])
                nc.sync.dma_start(out=dbg_aop[:], in_=aop[:])

            # ---- output projection (full contraction, T/2 own rows) ----
            for tk in range(TH // P if en("outproj") else 0):
                outs = outs_pool.tile([P, C], F32, tag="outs")
                for n in range(C // TJ):
                    ops = pp.tile([P, TJ], F32, tag="pp")
                    for jj in range(8):
                        blk, pjj = jj // 4, jj % 4
                        nc.tensor.matmul(
                            ops[:], aop[:, blk, pjj, tk * P:(tk + 1) * P],
                            wot_sb[:, jj, n * TJ:(n + 1) * TJ],
                            start=(jj == 0), stop=(jj == 7))
                    nc.vector.tensor_tensor(
                        out=outs[:, n * TJ:(n + 1) * TJ], in0=ops[:],
                        in1=bo_sb[:, n * TJ:(n + 1) * TJ],
                        op=mybir.AluOpType.add)
                nc.sync.dma_start(out=out_d[tk * P:(tk + 1) * P, :],
                                  in_=outs[:])

    nc.compile()
    _NC_CACHE[key] = nc
    return nc


def shard_inputs(x, Wq, Wk, Wv, Wo, bo):
    """Build the 8 per-core input maps."""
    x = np.asarray(x, np.float32)
    Wq = np.asarray(Wq, np.float32)
    Wk = np.asarray(Wk, np.float32)
    Wv = np.asarray(Wv, np.float32)
    Wo = np.asarray(Wo, np.float32)
    bo = np.asarray(bo, np.float32)
    bf = ml_dtypes.bfloat16
    wot = Wo.T.reshape(8, P, C).transpose(1, 0, 2)        # [P, 8, C]
    wot = np.ascontiguousarray(wot).astype(bf)
    bo_bc = np.ascontiguousarray(np.tile(bo, (P, 1)))
    m = np.arange(4)
    maskp = (np.arange(P)[:, None, None]
             <= (np.arange(TJ)[None, None, :] - P * m[None, :, None]))
    maskp = np.ascontiguousarray(maskp).astype(bf)        # [P, 4, TJ]
    in_maps = []
    for c in range(N_CORES):
        b, p = divmod(c, 2)
        hs = slice(p * NH, (p + 1) * NH)
        xb_ = np.ascontiguousarray(
            x[b].T.reshape(NCK, P, T).transpose(1, 0, 2)).astype(bf)
        wqk = np.concatenate([Wq[hs], Wk[hs]], axis=-1)       # [NH, C, 128]
        wqk = wqk.reshape(NH, NCK, P, P).transpose(2, 0, 1, 3)
        wv = Wv[hs].transpose(1, 0, 2).reshape(NCK, P, NH * HS)
        wv = wv.transpose(1, 0, 2)                            # [P, NCK, 512]
        in_maps.append({
            "xb": xb_,
            "wqk": np.ascontiguousarray(wqk).astype(bf),
            "wv": np.ascontiguousarray(wv).astype(bf),
            "wot": wot,
            "bo_bc": bo_bc,
            "maskp": maskp,
        })
    return in_maps


def gather_outputs(results):
    out = np.empty((B, T, C), np.float32)
    for c in range(N_CORES):
        b, r = divmod(c, 2)
        out[b, r * TH:(r + 1) * TH, :] = results[c]["out"]
    return out


def kernel(x, Wq, Wk, Wv, Wo, bo):
    nc = build_nc(with_collective=True)
    in_maps = shard_inputs(x, Wq, Wk, Wv, Wo, bo)
    res = run_bass_kernel_spmd(nc, in_maps, core_ids=list(range(N_CORES)))
    return gather_outputs(res.results)


_RUNNER_CACHE = {}


def _make_runner(nc, n_cores=N_CORES):
    """A jit-once SPMD runner mirroring bass2jax.run_bass_via_pjrt so that
    repeated executions can be timed without re-tracing."""
    if id(nc) in _RUNNER_CACHE:
        return _RUNNER_CACHE[id(nc)]
    import jax
    from jax.sharding import Mesh, PartitionSpec
    from jax.experimental.shard_map import shard_map
    from concourse import bass2jax

    bass2jax.install_neuronx_cc_hook()
    partition_name = (nc.partition_id_tensor.name
                      if nc.partition_id_tensor else None)
    in_names, out_names, out_avals, zero_outs = [], [], [], []
    for alloc in nc.m.functions[0].allocations:
        if not isinstance(alloc, mybir.MemoryLocationSet):
            continue
        name = alloc.memorylocations[0].name
        if alloc.kind == "ExternalInput":
            if name != partition_name:
                in_names.append(name)
        elif alloc.kind == "ExternalOutput":
            out_names.append(name)
            shape = tuple(alloc.tensor_shape)
            dtype = mybir.dt.np(alloc.dtype)
            out_avals.append(jax.core.ShapedArray(shape, dtype))
            zero_outs.append(np.zeros(shape, dtype))
    n_params = len(in_names)
    all_in = list(in_names) + list(out_names)
    if partition_name is not None:
        all_in.append(partition_name)
    donate = tuple(range(n_params, n_params + len(out_names)))

    def _body(*args):
        operands = list(args)
        if partition_name is not None:
            operands.append(bass2jax.partition_id_tensor())
        outs = bass2jax._bass_exec_p.bind(
            *operands,
            out_avals=tuple(out_avals),
            in_names=tuple(all_in),
            out_names=tuple(out_names),
            lowering_input_output_aliases=(),
            sim_require_finite=True,
            sim_require_nnan=True,
            nc=nc,
        )
        return tuple(outs)

    devices = jax.devices()[:n_cores]
    mesh = Mesh(np.asarray(devices), ("core",))
    in_specs = (PartitionSpec("core"),) * (n_params + len(out_names))
    out_specs = (PartitionSpec("core"),) * len(out_names)
    sharded = jax.jit(
        shard_map(_body, mesh=mesh, in_specs=in_specs, out_specs=out_specs,
                  check_rep=False),
        donate_argnums=donate, keep_unused=True)
    ret = (sharded, in_names, out_names, zero_outs, n_params)
    _RUNNER_CACHE[id(nc)] = ret
    return ret


def run_pjrt(in_maps, nc=None, iters=1):
    """Run the SPMD program via a persistent jitted callable; returns
    (per-core results, list of per-iteration wall times)."""
    import jax
    if nc is None:
        nc = build_nc(with_collective=True)
    sharded, in_names, out_names, zero_outs, n_params = _make_runner(nc)
    n_cores = len(in_maps)
    concat_in = [
        np.concatenate([np.asarray(in_maps[c][k]) for c in range(n_cores)],
                       axis=0)
        for k in in_names]
    concat_in = [jax.device_put(a) for a in concat_in]
    concat_in = jax.block_until_ready(concat_in)
    out_arrs = None
    times = []
    for _ in range(max(1, iters)):
        zeros = [jax.device_put(
            np.zeros((n_cores * z.shape[0], *z.shape[1:]), z.dtype))
            for z in zero_outs]
        zeros = jax.block_until_ready(zeros)
        t0 = time.perf_counter()
        out_arrs = jax.block_until_ready(sharded(*concat_in, *zeros))
        times.append(time.perf_counter() - t0)
    results = [
        {name: np.asarray(out_arrs[i]).reshape(
            n_cores, *(zero_outs[i].shape))[c]
         for i, name in enumerate(out_names)}
        for c in range(n_cores)]
    return results, times


def time_kernel(inputs, iters=6):
    in_maps = shard_inputs(**inputs)
    _, times = run_pjrt(in_maps, iters=iters)
    return times


if __name__ == "__main__":
    rng = np.random.default_rng(0)
    s = 0.02
    x = rng.standard_normal((B, T, C), dtype=np.float32)
    Wq = rng.standard_normal((H, C, HS), dtype=np.float32) * s
    Wk = rng.standard_normal((H, C, HS), dtype=np.float32) * s
    Wv = rng.standard_normal((H, C, HS), dtype=np.float32) * s
    Wo = rng.standard_normal((C, C), dtype=np.float32) * s
    bo = np.zeros((C,), np.float32)
    got = kernel(x, Wq, Wk, Wv, Wo, bo)
    print("ran", got.shape, got.dtype)


# revision 39
# speedup vs baseline: 1.1144x; 1.1144x over previous
"""Trainium2 Bass kernel: multi-head causal attention (B=4, T=2048, C=1024, H=16, HS=64).

Sharding: hybrid batch x head tensor-parallel over 8 cores.
  core c -> batch b = c//2, head half p = c%2 (heads p*8 .. p*8+8).
Each core computes Q/K/V projections for its 8 heads on its batch and causal
flash-style attention (scores transposed: s on partitions, t on free dim,
softmax denominator via a ones-column appended to V).  The attention outputs
(ao, bf16) are exchanged between the two cores of a batch pair with four
small per-head-pair AllToAlls (each core keeps the t-half it will project),
then each core runs the full-contraction output projection for its T/2 rows.
No ReduceScatter of f32 partials is needed.

Perf notes vs the earlier RS-based version:
  - reciprocal batched over 8 denominator rows at once ([8,512] costs the
    same as [1,512]; DVE RECIPROCAL is ~8 cycles/element along the free dim)
  - exp batched in 2-PSUM-bank pairs (amortizes the ~352-cycle ACT ramp)
  - qt/kt half duplication (for PE row tiling) done by SBUF->SBUF DMA
    instead of extra DVE copies
  - causal masks for a chunk pair applied in one tensor_tensor op against a
    host-precomputed [P, 4, 512] mask
"""

import os
import sys
import time

import numpy as np

for _p in ("/opt/trn_rl_repo", "/root/.axon_site/_ro/trn_rl_repo"):
    if os.path.isdir(_p) and _p not in sys.path:
        sys.path.insert(0, _p)

import ml_dtypes  # noqa: E402
import concourse.bass as bass  # noqa: E402,F401
import concourse.mybir as mybir  # noqa: E402
import concourse.tile as tile  # noqa: E402
from concourse import bacc  # noqa: E402
from concourse.bass_utils import run_bass_kernel_spmd  # noqa: E402

B, T, C, H, HS = 4, 2048, 1024, 16, 64
N_CORES = 8
NH = H // 2          # heads per core
P = 128
TJ = 512             # t-tile width for attention
NTJ = T // TJ        # 4
NSK = T // P         # 16 s-chunks
NCK = C // P         # 8 contraction chunks
TH = T // 2          # rows output per core
BF16 = mybir.dt.bfloat16
F32 = mybir.dt.float32
GROUPS = [[0, 1], [2, 3], [4, 5], [6, 7]]

# Schraudolph bf16-exp constants: int16(bf16 bits of exp(score/sqrt(HS)))
# ~= score * (2^7 * log2(e) / sqrt(HS)) + (127*2^7 - 5.58)
_SCH_A = 128.0 * 1.4426950408889634 / 8.0
_SCH_B = 16250.4

_NC_CACHE = {}


def build_nc(with_collective=True, stages=frozenset(
        {"proj", "scores", "exp", "mask", "av", "norm", "outproj", "cc"}),
        repeat=1, sp_bufs=2, avp_bufs=2, dup_dma=True, dve_every=0):
    key = (with_collective, tuple(sorted(stages)), repeat, sp_bufs, avp_bufs,
           dup_dma, dve_every)
    if key in _NC_CACHE:
        return _NC_CACHE[key]
    en = stages.__contains__
    nc = bacc.Bacc("TRN2", target_bir_lowering=False, debug=False,
                   num_devices=N_CORES)
    xb_d = nc.dram_tensor("xb", [P, NCK, T], BF16, kind="ExternalInput")
    wqk_d = nc.dram_tensor("wqk", [P, NH, NCK, P], BF16, kind="ExternalInput")
    wv_d = nc.dram_tensor("wv", [P, NCK, NH * HS], BF16, kind="ExternalInput")
    wot_d = nc.dram_tensor("wot", [P, 8, C], BF16, kind="ExternalInput")
    bo_d = nc.dram_tensor("bo_bc", [P, C], F32, kind="ExternalInput")
    mask_d = nc.dram_tensor("maskp", [P, 4, TJ], BF16, kind="ExternalInput")
    use_cc = with_collective and en("cc")
    if use_cc:
        ccin = [nc.dram_tensor(f"ccin{pj}", [P, T], BF16)
                for pj in range(4)]
        ccout = [nc.dram_tensor(f"ccout{pj}", [2 * P, T], BF16)
                 for pj in range(4)]
    out_d = nc.dram_tensor("out", [TH, C], F32, kind="ExternalOutput")
    if en("dbg"):
        dbg_ao = nc.dram_tensor("dbg_ao", [P, 4, T], BF16,
                                kind="ExternalOutput")
        dbg_aop = nc.dram_tensor("dbg_aop", [P, 2, 4, TH], BF16,
                                 kind="ExternalOutput")

    from contextlib import ExitStack
    with tile.TileContext(nc) as tc, ExitStack() as ctx:
        const = ctx.enter_context(tc.tile_pool(name="const", bufs=1))
        qk_pool = ctx.enter_context(tc.tile_pool(name="qk", bufs=2))
        attn_pool = ctx.enter_context(tc.tile_pool(name="attn", bufs=2))
        den_pool = ctx.enter_context(tc.tile_pool(name="den", bufs=2))
        stash_pool = ctx.enter_context(tc.tile_pool(name="stash", bufs=1))
        outs_pool = ctx.enter_context(tc.tile_pool(name="outs", bufs=2))
        pp = ctx.enter_context(tc.tile_pool(name="pp", bufs=2, space="PSUM"))
        sp = ctx.enter_context(tc.tile_pool(name="sp", bufs=sp_bufs, space="PSUM"))
        avp = ctx.enter_context(tc.tile_pool(name="avp", bufs=avp_bufs, space="PSUM"))

        # ---- constants / persistent tiles ----
        wqk_sb = const.tile([P, NH, NCK, P], BF16)
        wv_sb = const.tile([P, NCK, NH * HS], BF16)
        wot_sb = const.tile([P, 8, C], BF16)
        bo_sb = const.tile([P, C], F32)
        xb = const.tile([P, NCK, T], BF16)          # x^T bf16 (c, t)
        v1 = const.tile([P, NH, NSK, HS + 1], BF16)  # [s, head, sk, d|1]
        ao = const.tile([P, 4, T], BF16)            # normalized attn out^T
        aop = const.tile([P, 2, 4, TH], BF16)       # A2A result (blk, pj, t)
        mask_sb = const.tile([P, 4, TJ], BF16)
        for i4 in range(4):
            nc.sync.dma_start(out=wqk_sb[:, 2 * i4:2 * i4 + 2, :, :],
                              in_=wqk_d[:, 2 * i4:2 * i4 + 2, :, :])
            nc.sync.dma_start(out=wot_sb[:, 2 * i4:2 * i4 + 2, :],
                              in_=wot_d[:, 2 * i4:2 * i4 + 2, :])
        for i2 in range(2):
            nc.sync.dma_start(out=wv_sb[:, 4 * i2:4 * i2 + 4, :],
                              in_=wv_d[:, 4 * i2:4 * i2 + 4, :])
        nc.sync.dma_start(out=bo_sb[:], in_=bo_d[:])
        nc.sync.dma_start(out=mask_sb[:], in_=mask_d[:])
        nc.vector.memset(v1[:, :, :, HS:HS + 1], 1.0)

        if use_cc:
            # rank within the pair, as a sync-engine register for
            # predicated (cond=) DMAs -- the program is SPMD, so any
            # rank-dependent addressing must be runtime-predicated.
            _pid = nc.sync.partition_id()
            r_reg = _pid % 2
            nr_reg = 1 - r_reg

        def emit_outproj():
            # full-contraction output projection for this core's T/2 rows,
            # reading the AllGathered attention outputs of the last-emitted
            # attention pass
            for tk in range(TH // P if en("outproj") else 0):
                outs = outs_pool.tile([P, C], F32, tag="outs")
                for n in range(C // TJ):
                    ops = pp.tile([P, TJ], F32, tag="pp")
                    for jj in range(8):
                        blk, pjj = jj // 4, jj % 4
                        nc.tensor.matmul(
                            ops[:], aop[:, blk, pjj, tk * P:(tk + 1) * P],
                            wot_sb[:, jj, n * TJ:(n + 1) * TJ],
                            start=(jj == 0), stop=(jj == 7))
                    nc.vector.tensor_tensor(
                        out=outs[:, n * TJ:(n + 1) * TJ], in0=ops[:],
                        in1=bo_sb[:, n * TJ:(n + 1) * TJ],
                        op=mybir.AluOpType.add)
                nc.sync.dma_start(out=out_d[tk * P:(tk + 1) * P, :],
                                  in_=outs[:])

        for _rep in range(repeat):

            # ---- load x^T (bf16, split for DMA-engine parallelism) ----
            for h in range(2):
                for ck in range(NCK):
                    nc.sync.dma_start(
                        out=xb[:, ck, h * TH:(h + 1) * TH],
                        in_=xb_d[:, ck, h * TH:(h + 1) * TH])

            # ---- V projection (all heads at once) ----
            for sk in range(NSK if en("proj") else 0):
                ps = pp.tile([P, NH * HS], F32, tag="pp")
                for ck in range(NCK):
                    nc.tensor.matmul(
                        ps[:], xb[:, ck, sk * P:(sk + 1) * P], wv_sb[:, ck, :],
                        start=(ck == 0), stop=(ck == NCK - 1))
                nc.vector.tensor_copy(
                    v1[:, :, sk, 0:HS],
                    ps[:].rearrange("p (i d) -> p i d", d=HS))

            # ---- output projection of the PREVIOUS repetition ----
            # Emitted here (after this rep's V projection) so the PE has
            # dense work to chew on while the previous rep's last AllGather
            # completes; removes a ~40us all-engines-idle rep boundary.
            if _rep > 0:
                emit_outproj()

            # ---- per-head attention, software-pipelined ----
            # PE is in-order; scores matmuls are paced by ACT exp draining
            # PSUM slots.  To keep PE busy during those waits, the previous
            # block's AV matmuls (which read SBUF attn tiles and their own
            # PSUM bank) are interleaved between scores/proj matmuls.
            def make_av_thunks(i, tj, attn, stash):
                if not en("av"):
                    return []
                pj = i // 2
                half = (i % 2) * HS
                n_sk = 4 * (tj + 1)
                av = avp.tile([P, TJ], F32, tag="avp")

                def mm(sk):
                    nc.tensor.matmul(av[0:HS + 1, :], v1[:, i, sk, :],
                                     attn[:, sk, :], start=(sk == 0),
                                     stop=(sk == n_sk - 1))
                thunks = [lambda sk=sk: mm(sk) for sk in range(n_sk)]

                def evac():
                    if not en("norm"):
                        return
                    if en("oldnorm"):
                        recip = den_pool.tile([1, TJ], F32, tag="recip1")
                        nc.vector.reciprocal(recip[:], av[HS:HS + 1, :])
                        den_bc = den_pool.tile([HS, TJ], F32, tag="den_bc0")
                        nc.gpsimd.partition_broadcast(den_bc[:], recip[:])
                        nc.vector.tensor_tensor(
                            out=ao[half:half + HS, pj, tj * TJ:(tj + 1) * TJ],
                            in0=av[0:HS, :], in1=den_bc[:],
                            op=mybir.AluOpType.mult)
                        return
                    # unnormalized attention output + denominator row stash
                    # (packed on the free dim of a partition-0 tile; DVE and
                    # gpsimd only reliably address partition bases 0/64)
                    nc.vector.tensor_copy(
                        ao[half:half + HS, pj, tj * TJ:(tj + 1) * TJ],
                        av[0:HS, :])
                    nc.vector.tensor_copy(
                        stash[0:1, (i % 2) * NTJ + tj, :],
                        av[HS:HS + 1, :])
                thunks.append(evac)
                return thunks

            def make_norm_thunks(pj, stash):
                # after a head pair: spread the eight stashed denominator
                # rows across partitions via DMA, one batched reciprocal,
                # then scale ao in place.  Only partition bases 0/64 are
                # used for compute-engine APs (others are unreliable on HW).
                if not en("norm") or en("oldnorm"):
                    return []
                thunks = []
                st2d = den_pool.tile([32, TJ], BF16, tag="st2d")
                recip8 = den_pool.tile([32, TJ], BF16, tag="recip")

                def recip_all():
                    nc.sync.dma_start(out=st2d[0:2 * NTJ, :],
                                      in_=stash[0:1, :, :])
                    with nc.allow_low_precision(
                            reason="bf16 softmax denominators"):
                        nc.vector.reciprocal(recip8[:], st2d[:])
                thunks.append(recip_all)

                def norm_one(hh, tj):
                    half = hh * HS
                    tjsl = slice(tj * TJ, (tj + 1) * TJ)
                    r1 = den_pool.tile([1, TJ], BF16, tag="r1")
                    nc.sync.dma_start(
                        out=r1[:], in_=recip8[hh * NTJ + tj:hh * NTJ + tj + 1, :])
                    den_bc = den_pool.tile([P, TJ], BF16, tag="den_bc")
                    nc.gpsimd.partition_broadcast(den_bc[:], r1[:])
                    nc.vector.tensor_tensor(
                        out=ao[half:half + HS, pj, tjsl],
                        in0=ao[half:half + HS, pj, tjsl],
                        in1=den_bc[half:half + HS, :],
                        op=mybir.AluOpType.mult)
                for hh in range(2):
                    for tj in range(NTJ):
                        thunks.append(
                            lambda hh=hh, tj=tj: norm_one(hh, tj))
                return thunks

            def make_fin_thunks(pj):
                # after both heads of pair pj: exchange halves with the pair
                thunks = []
                if use_cc:
                    def exchange():
                        nc.sync.dma_start(out=ccin[pj][:], in_=ao[:, pj, :])
                        nc.gpsimd.collective_compute(
                            "AllGather", mybir.AluOpType.bypass,
                            replica_groups=GROUPS,
                            ins=[ccin[pj][:]], outs=[ccout[pj][:]])
                        # each rank keeps the t-half it will project
                        for blk in range(2):
                            rows = slice(blk * P, (blk + 1) * P)
                            if en("recvlo"):
                                nc.sync.dma_start(
                                    out=aop[:, blk, pj, :],
                                    in_=ccout[pj][rows, 0:TH])
                            else:
                                nc.sync.dma_start(
                                    out=aop[:, blk, pj, :],
                                    in_=ccout[pj][rows, 0:TH], cond=nr_reg)
                                nc.sync.dma_start(
                                    out=aop[:, blk, pj, :],
                                    in_=ccout[pj][rows, TH:T], cond=r_reg)
                    thunks.append(exchange)
                return thunks

            def emit_block(emitters, pending):
                L_s, L_a = len(emitters), len(pending)
                j = 0
                for k, e in enumerate(emitters):
                    e()
                    jt = L_a * (k + 1) // L_s if L_s else L_a
                    while j < jt:
                        pending[j]()
                        j += 1
                for t in pending[j:]:
                    t()

            pending = []
            _gp = [0]
            stash = None
            for i in range(NH):
                if i % 2 == 0:
                    stash = stash_pool.tile([1, 2 * NTJ, TJ], BF16,
                                            tag="stash")
                qt2 = qk_pool.tile([P, T], BF16, tag="qt2")
                kt2 = qk_pool.tile([P, T], BF16, tag="kt2")

                proj_emitters = []
                if en("proj"):
                    for tj in range(NTJ):
                        ps = pp.tile([P, TJ], F32, tag="pp")

                        def pmm(i=i, tj=tj, ps=ps, ck=None):
                            nc.tensor.matmul(
                                ps[:], wqk_sb[:, i, ck, :],
                                xb[:, ck, tj * TJ:(tj + 1) * TJ],
                                start=(ck == 0), stop=(ck == NCK - 1))

                        def pevac(i=i, tj=tj, ps=ps, qt2=qt2, kt2=kt2):
                            sl = slice(tj * TJ, (tj + 1) * TJ)
                            nc.vector.tensor_copy(qt2[0:HS, sl], ps[0:HS, :])
                            nc.vector.tensor_copy(kt2[HS:P, sl], ps[HS:P, :])
                            if dup_dma:
                                nc.sync.dma_start(out=qt2[HS:P, sl],
                                                  in_=qt2[0:HS, sl])
                                nc.sync.dma_start(out=kt2[0:HS, sl],
                                                  in_=kt2[HS:P, sl])
                            else:
                                nc.vector.tensor_copy(qt2[HS:P, sl],
                                                      ps[0:HS, :])
                                nc.vector.tensor_copy(kt2[0:HS, sl],
                                                      ps[HS:P, :])
                        for ck in range(NCK):
                            proj_emitters.append(
                                lambda f=pmm, ck=ck: f(ck=ck))
                        proj_emitters.append(pevac)
                emit_block(proj_emitters, pending)
                pending = []

                for tj in range(NTJ):
                    n_sk = 4 * (tj + 1)
                    attn = attn_pool.tile([P, n_sk, TJ], BF16, tag="attn")
                    sc_emitters = []
                    if en("nopair"):
                        for sk in range(n_sk if en("scores") else 0):
                            def smm1(i=i, tj=tj, sk=sk, attn=attn):
                                h0 = HS * (sk % 2)
                                sps = sp.tile([P, TJ], F32, tag="sp")
                                nc.tensor.matmul(
                                    sps[:],
                                    kt2[h0:h0 + HS, sk * P:(sk + 1) * P],
                                    qt2[h0:h0 + HS, tj * TJ:(tj + 1) * TJ],
                                    start=True, stop=True)
                                if en("exp"):
                                    nc.scalar.activation(
                                        attn[:, sk, :], sps[:],
                                        mybir.ActivationFunctionType.Exp,
                                        scale=1.0 / np.sqrt(HS))
                                kdiag = sk - 4 * tj
                                if kdiag >= 0 and en("mask"):
                                    nc.vector.tensor_tensor(
                                        out=attn[:, sk, :],
                                        in0=attn[:, sk, :],
                                        in1=mask_sb[:, kdiag, :],
                                        op=mybir.AluOpType.mult)
                            sc_emitters.append(smm1)
                    else:
                        # scores in pairs sharing one 2-bank PSUM tile; one
                        # exp per pair (amortizes ACT ramp); the diagonal
                        # region (last 4 chunks) gets one merged mask mult.
                        # A fraction of exps runs on DVE via the Schraudolph
                        # bit trick to offload the saturated ACT engine.
                        for p2 in range(n_sk // 2 if en("scores") else 0):
                            sk0 = 2 * p2
                            sps = sp.tile([P, 2, TJ], F32, tag="sp")
                            _gp[0] += 1
                            on_dve = dve_every and (_gp[0] % dve_every == 0)

                            def smm2(i=i, tj=tj, sk0=sk0, sps=sps):
                                for j in range(2):
                                    sk = sk0 + j
                                    h0 = HS * (sk % 2)
                                    nc.tensor.matmul(
                                        sps[:, j, :],
                                        kt2[h0:h0 + HS, sk * P:(sk + 1) * P],
                                        qt2[h0:h0 + HS,
                                            tj * TJ:(tj + 1) * TJ],
                                        start=True, stop=True)

                            def sexp(i=i, tj=tj, sk0=sk0, sps=sps, attn=attn,
                                     on_dve=on_dve):
                                if en("exp"):
                                    if on_dve:
                                        # bf16(exp(x*scale)) bit pattern as
                                        # int16 affine of the raw score
                                        nc.vector.tensor_scalar(
                                            out=attn[:, sk0:sk0 + 2, :]
                                            .bitcast(mybir.dt.int16),
                                            in0=sps[:],
                                            scalar1=_SCH_A, scalar2=_SCH_B,
                                            op0=mybir.AluOpType.mult,
                                            op1=mybir.AluOpType.add)
                                    else:
                                        nc.scalar.activation(
                                            attn[:, sk0:sk0 + 2, :], sps[:],
                                            mybir.ActivationFunctionType.Exp,
                                            scale=1.0 / np.sqrt(HS))
                                if sk0 == 4 * tj + 2 and en("mask"):
                                    nc.vector.tensor_tensor(
                                        out=attn[:, 4 * tj:4 * tj + 4, :],
                                        in0=attn[:, 4 * tj:4 * tj + 4, :],
                                        in1=mask_sb[:],
                                        op=mybir.AluOpType.mult)
                            sc_emitters.append(smm2)
                            sc_emitters.append(sexp)
                    emit_block(sc_emitters, pending)
                    pending = make_av_thunks(i, tj, attn, stash)
                if i % 2 == 1:
                    pending = (pending + make_norm_thunks(i // 2, stash)
                               + make_fin_thunks(i // 2))
            for t in pending:
                t()
            pending = []

            if en("dbg"):
                nc.sync.dma_start(out=dbg_ao[:], in_=ao[:])
                nc.sync.dma_start(out=dbg_aop[:], in_=aop[:])

        # output projection of the final repetition
        emit_outproj()

    nc.compile()
    _NC_CACHE[key] = nc
    return nc


def shard_inputs(x, Wq, Wk, Wv, Wo, bo):
    """Build the 8 per-core input maps."""
    x = np.asarray(x, np.float32)
    Wq = np.asarray(Wq, np.float32)
    Wk = np.asarray(Wk, np.float32)
    Wv = np.asarray(Wv, np.float32)
    Wo = np.asarray(Wo, np.float32)
    bo = np.asarray(bo, np.float32)
    bf = ml_dtypes.bfloat16
    wot = Wo.T.reshape(8, P, C).transpose(1, 0, 2)        # [P, 8, C]
    wot = np.ascontiguousarray(wot).astype(bf)
    bo_bc = np.ascontiguousarray(np.tile(bo, (P, 1)))
    m = np.arange(4)
    maskp = (np.arange(P)[:, None, None]
             <= (np.arange(TJ)[None, None, :] - P * m[None, :, None]))
    maskp = np.ascontiguousarray(maskp).astype(bf)        # [P, 4, TJ]
    in_maps = []
    for c in range(N_CORES):
        b, p = divmod(c, 2)
        hs = slice(p * NH, (p + 1) * NH)
        xb_ = np.ascontiguousarray(
            x[b].T.reshape(NCK, P, T).transpose(1, 0, 2)).astype(bf)
        wqk = np.concatenate([Wq[hs], Wk[hs]], axis=-1)       # [NH, C, 128]
        wqk = wqk.reshape(NH, NCK, P, P).transpose(2, 0, 1, 3)
        wv = Wv[hs].transpose(1, 0, 2).reshape(NCK, P, NH * HS)
        wv = wv.transpose(1, 0, 2)                            # [P, NCK, 512]
        in_maps.append({
            "xb": xb_,
            "wqk": np.ascontiguousarray(wqk).astype(bf),
            "wv": np.ascontiguousarray(wv).astype(bf),
            "wot": wot,
            "bo_bc": bo_bc,
            "maskp": maskp,
        })
    return in_maps


def gather_outputs(results):
    out = np.empty((B, T, C), np.float32)
    for c in range(N_CORES):
        b, r = divmod(c, 2)
        out[b, r * TH:(r + 1) * TH, :] = results[c]["out"]
    return out


def kernel(x, Wq, Wk, Wv, Wo, bo):
    nc = build_nc(with_collective=True)
    in_maps = shard_inputs(x, Wq, Wk, Wv, Wo, bo)
    res = run_bass_kernel_spmd(nc, in_maps, core_ids=list(range(N_CORES)))
    return gather_outputs(res.results)


_RUNNER_CACHE = {}


def _make_runner(nc, n_cores=N_CORES):
    """A jit-once SPMD runner mirroring bass2jax.run_bass_via_pjrt so that
    repeated executions can be timed without re-tracing."""
    if id(nc) in _RUNNER_CACHE:
        return _RUNNER_CACHE[id(nc)]
    import jax
    from jax.sharding import Mesh, PartitionSpec
    from jax.experimental.shard_map import shard_map
    from concourse import bass2jax

    bass2jax.install_neuronx_cc_hook()
    partition_name = (nc.partition_id_tensor.name
                      if nc.partition_id_tensor else None)
    in_names, out_names, out_avals, zero_outs = [], [], [], []
    for alloc in nc.m.functions[0].allocations:
        if not isinstance(alloc, mybir.MemoryLocationSet):
            continue
        name = alloc.memorylocations[0].name
        if alloc.kind == "ExternalInput":
            if name != partition_name:
                in_names.append(name)
        elif alloc.kind == "ExternalOutput":
            out_names.append(name)
            shape = tuple(alloc.tensor_shape)
            dtype = mybir.dt.np(alloc.dtype)
            out_avals.append(jax.core.ShapedArray(shape, dtype))
            zero_outs.append(np.zeros(shape, dtype))
    n_params = len(in_names)
    all_in = list(in_names) + list(out_names)
    if partition_name is not None:
        all_in.append(partition_name)
    donate = tuple(range(n_params, n_params + len(out_names)))

    def _body(*args):
        operands = list(args)
        if partition_name is not None:
            operands.append(bass2jax.partition_id_tensor())
        outs = bass2jax._bass_exec_p.bind(
            *operands,
            out_avals=tuple(out_avals),
            in_names=tuple(all_in),
            out_names=tuple(out_names),
            lowering_input_output_aliases=(),
            sim_require_finite=True,
            sim_require_nnan=True,
            nc=nc,
        )
        return tuple(outs)

    devices = jax.devices()[:n_cores]
    mesh = Mesh(np.asarray(devices), ("core",))
    in_specs = (PartitionSpec("core"),) * (n_params + len(out_names))
    out_specs = (PartitionSpec("core"),) * len(out_names)
    sharded = jax.jit(
        shard_map(_body, mesh=mesh, in_specs=in_specs, out_specs=out_specs,
                  check_rep=False),
        donate_argnums=donate, keep_unused=True)
    ret = (sharded, in_names, out_names, zero_outs, n_params)
    _RUNNER_CACHE[id(nc)] = ret
    return ret


def run_pjrt(in_maps, nc=None, iters=1):
    """Run the SPMD program via a persistent jitted callable; returns
    (per-core results, list of per-iteration wall times)."""
    import jax
    if nc is None:
        nc = build_nc(with_collective=True)
    sharded, in_names, out_names, zero_outs, n_params = _make_runner(nc)
    n_cores = len(in_maps)
    concat_in = [
        np.concatenate([np.asarray(in_maps[c][k]) for c in range(n_cores)],
                       axis=0)
        for k in in_names]
    concat_in = [jax.device_put(a) for a in concat_in]
    concat_in = jax.block_until_ready(concat_in)
    out_arrs = None
    times = []
    for _ in range(max(1, iters)):
        zeros = [jax.device_put(
            np.zeros((n_cores * z.shape[0], *z.shape[1:]), z.dtype))
            for z in zero_outs]
        zeros = jax.block_until_ready(zeros)
        t0 = time.perf_counter()
        out_arrs = jax.block_until_ready(sharded(*concat_in, *zeros))
        times.append(time.perf_counter() - t0)
    results = [
        {name: np.asarray(out_arrs[i]).reshape(
            n_cores, *(zero_outs[i].shape))[c]
         for i, name in enumerate(out_names)}
        for c in range(n_cores)]
    return results, times


def time_kernel(inputs, iters=6):
    in_maps = shard_inputs(**inputs)
    _, times = run_pjrt(in_maps, iters=iters)
    return times


if __name__ == "__main__":
    rng = np.random.default_rng(0)
    s = 0.02
    x = rng.standard_normal((B, T, C), dtype=np.float32)
    Wq = rng.standard_normal((H, C, HS), dtype=np.float32) * s
    Wk = rng.standard_normal((H, C, HS), dtype=np.float32) * s
    Wv = rng.standard_normal((H, C, HS), dtype=np.float32) * s
    Wo = rng.standard_normal((C, C), dtype=np.float32) * s
    bo = np.zeros((C,), np.float32)
    got = kernel(x, Wq, Wk, Wv, Wo, bo)
    print("ran", got.shape, got.dtype)
